# revision 13
# baseline (speedup 1.0000x reference)
"""Trainium2 Bass kernel for BinaryLinearUnit:
    y = sign(x) @ sign(w).T ; BatchNorm1d(train) ; * gamma + beta

Strategy: 2D sharding over 8 NeuronCores — 4 batch shards x 2
output-feature shards. Each core computes y.T for its [2048 batch x
2048 out-features] block with an FP8 (DoubleRow) matmul.

Signs are precomputed on the host (sharding-time byte maps, exact):
  - x ships as +-0.5 fp8e4m3 bytes (0x30/0xB0), K-major: 8MB/core.
    BatchNorm cancels any constant scale of y, so +-0.5 == +-1.
  - w ships as +-1 fp8e4m3 bytes (0x38/0xB8), K-major packed: 8MB/core.
  - y_hat ships back as fp16 (~5e-4 rel err): 8MB/core.
PSUM accumulation is fp32-exact (y/2 is a sum of +-0.5 with |y|<=4096,
and y/2 is exact in fp16). No sign ops on device at all — ACT and DVE
are free for BN work, and the first matmul is gated only by the first
w/x DMA chunks.

BN batch stats need cross-core reduction only within each group of 4
cores that shares the same output-feature shard (logical groups
[0-3], [4-7] — XOR-cosets). Instead of collective_compute AllGathers
(~31us each on the CC stream, measured), partial [mean, E[y^2]] tiles
are exchanged with remote_dma_broadcast: each core sends its partial
to peer c^k which lands in slot k of the peer's SBUF landing tile
(XOR symmetry makes the same static APs correct on every core), with
a monotonic-semaphore bump on arrival. Exchange latency is a few us,
so the post-matmul tail is bn_stats + exchange + normalize (~15us)
instead of a ~45us exposed collective. Descriptors are pre-generated
on GpSimd well before each group ends; only trigger_dma is on the
critical path (the Tile-managed count=None path carries the source
read deps).

The output tiles are processed in NSPLIT stat groups; earlier groups'
exchanges + post math + normalization all overlap remaining matmuls.
The PE span is throttle-bound (GPIO 13/16 duty after ~40us — board
power limit, measured via ntff ham records), so the only other levers
are the startup to first matmul and the tail.

Engine assignment: PE matmuls | ACT x-input DMA queue + sqrt + half
the output stores | DVE bn_stats, psum->f16 copy, stats math, rdma
waits, normalize | GpSimd rdma desc-gen + triggers | Sync w/gb DMA +
half the output stores.
"""

import numpy as np
import ml_dtypes

import concourse.bass as bass
import concourse.mybir as mybir
import concourse.tile as tile
import concourse.bass_interp as bass_interp
from concourse import bacc
from concourse.bass import ts, create_sync_update
from concourse.bass_utils import run_bass_kernel_spmd
from concourse.tile_rust import add_dep_helper

N_CORES = 8
KB_SHARD = 4            # batch shards
KO_SHARD = 2            # output-feature shards
BN_EPS = 1e-5

f32 = mybir.dt.float32
f16 = mybir.dt.float16
fp8 = mybir.dt.float8e4

# The Tile scheduling pass runs a single-core no-exec sim in which
# remote-DMA sem increments from peer cores never arrive, so a wait on
# them deadlocks the scheduler. Seed those sems with a huge value in
# the scheduling pass only; on HW the real wait still blocks until the
# peers' writes land.
_SEED_SEMS: list = []
_orig_simulate = bass_interp.CoreSim.simulate


def _sim_with_seed(self, *a, **k):
    if self.is_scheduling_pass():
        for h in _SEED_SEMS:
            self.update_semaphore(
                create_sync_update(h, 1 << 20, skip_validation=True)
            )
    return _orig_simulate(self, *a, **k)


bass_interp.CoreSim.simulate = _sim_with_seed


def build(B, IN, OUT, kb=KB_SHARD, ko=KO_SHARD):
    """Per-core SPMD module. Core c handles batch shard c%kb and
    out-feature shard c//kb. Shapes: x [B, IN], w [OUT, IN]."""
    Bc = B // kb            # batch rows per core
    OUTc = OUT // ko        # out features per core
    KT = IN // 128          # k tiles (contraction)
    KP = KT // 2            # fp8 DoubleRow consumes k-pairs
    OT = OUTc // 128        # output-feature tiles per core
    NB = 512                # matmul free dim / psum bank width
    BT = Bc // NB           # b tiles per core
    n_group = N_CORES // ko  # cores sharing one out-feature shard

    # Stat groups: each group's exchange (a few us) + post + normalize
    # overlap the remaining matmuls; only the last group's chain is an
    # exposed tail, so it is small.
    GS = [7, 7, 2] if OT == 16 else [OT - OT // 2, OT // 2]
    NSPLIT = len(GS)
    GO = [sum(GS[:q]) for q in range(NSPLIT)]

    nc = bacc.Bacc("TRN2", target_bir_lowering=False, debug=False,
                   num_devices=N_CORES, monotonic_sem_count=1)

    # Per-core external I/O (host pre-transposed, K-major, pre-signed):
    #   xt[k, b] = sign(x[(c%kb)*Bc + b, k]) * 0.5          fp8e4m3
    #   w2[ot, p, ks, o] = sign(w[(c//kb)*OUTc + ot*128 + o, ks*128 + p])
    #   yt[o, b] = out[(c%kb)*Bc + b, (c//kb)*OUTc + o]     fp16
    xt = nc.dram_tensor("xt", [IN, Bc], fp8, kind="ExternalInput")
    w2 = nc.dram_tensor("w2", [OT, 128, KT, 128], fp8, kind="ExternalInput")
    gb = nc.dram_tensor("gb", [128, 2, OT], f32, kind="ExternalInput")
    yt = nc.dram_tensor("yt", [OUTc, Bc], f16, kind="ExternalOutput")

    # Dummy 8-core collective: its presence makes the runtime build the
    # global comm and align core launches (without any collective the
    # cores free-run after their own input staging, ~1.6ms apart —
    # measured: peers' rdma arrived in ms-late bursts). Nothing consumes
    # its output; it runs on the CC stream concurrent with startup DMA.
    ccd_in = nc.dram_tensor("ccd_in", [128, 1], f32)
    ccd_out = nc.dram_tensor("ccd_out", [N_CORES * 128, 1], f32)

    # Monotonic sem bumped by peers' remote writes (+2 per peer per
    # group: 8 slots -> 16//8 increments per real dest). Same sem num
    # on every core (SPMD). lsem is the send-side release sem (unused:
    # source tiles are standing, never rewritten).
    rsem = nc.monotonic_semaphore(0).sem()
    _SEED_SEMS.clear()
    _SEED_SEMS.append(rsem)
    lsem = nc.alloc_semaphore("rdma_local")
    # Precise trigger gating: the arT-writing DVE ops bump tsem via
    # then_inc, and the GpSimd trigger waits on it directly. (A plain
    # cross-engine dep edge resolves via Tile's periodic engine-tick
    # sems — measured ~9us of slack on the critical trigger path.)
    tsem = nc.alloc_semaphore("rdma_trig")

    with tile.TileContext(nc) as tc:
        with (
            tc.tile_pool(name="big", bufs=1) as big,
            tc.tile_pool(name="sw", bufs=3) as swp,
            tc.tile_pool(name="ps", bufs=2, space="PSUM") as psp,
            tc.tile_pool(name="st", bufs=2) as stp,
            tc.tile_pool(name="outp", bufs=6) as outp,
        ):
            # Standing tensors
            sxT = big.tile([128, KT, Bc], fp8)          # sign(x)/2, K-major
            yTt = big.tile([128, OT, Bc], f16)          # y.T/2 (exact in fp16)
            mvT = big.tile([128, 2, OT], f32)           # per-core [mean, var]
            gbt = big.tile([128, 2, OT], f32)           # [gamma; beta]
            scal = big.tile([128, OT], f32)             # gamma * rstd
            nbias = big.tile([128, OT], f32)            # beta - mean * scal
            epsT = big.tile([128, 1], f32)              # BN eps / 4 (ACT bias)
            nc.vector.memset(epsT[:], BN_EPS / 4.0)
            # Exchange tiles (standing: remote reads/writes outlive any
            # pool-recycle window Tile could reason about).
            arTs = [big.tile([128, 2, GS[q]], f32, name=f"arT{q}")
                    for q in range(NSPLIT)]
            # grA slot 0 = own partial, slot k = from logical peer c^k.
            grAs = [big.tile([128, n_group, 2, GS[q]], f32, name=f"grA{q}")
                    for q in range(NSPLIT)]

            def rdma_prep(q):
                # Desc-gen for group q's three peer sends, emitted well
                # before the group ends so only trigger_dma is on the
                # critical path. Slot k of the receiver gets the sender
                # at XOR-distance k (same static APs on every core).
                for k in range(1, n_group):
                    rd = [None] * 8
                    rd[k] = (0, k)
                    nc.gpsimd.remote_dma_broadcast(
                        out_ap=grAs[q][:, k, :, :], in_ap=arTs[q][:],
                        remote_sem=rsem, local_sem=lsem, rdests=rd,
                    )

            def alloc_psums():
                return [
                    psp.tile([128, NB], f32, tag=f"ps{bt}", name=f"psum{bt}")
                    for bt in range(BT)
                ]

            def mm_mms(swt, psums):
                # kp-outer: each stationary load is reused across BT b-tiles;
                # also consumes the x k-pairs progressively during startup.
                for kp in range(KP):
                    for bt in range(BT):
                        nc.tensor.matmul(
                            psums[bt][:],
                            lhsT=swt[:, 2 * kp : 2 * kp + 2, :],
                            rhs=sxT[:, 2 * kp : 2 * kp + 2, ts(bt, NB)],
                            start=(kp == 0),
                            stop=(kp == KP - 1),
                            perf_mode=mybir.MatmulPerfMode.DoubleRow,
                        )

            def mm_fused01(swts):
                # Fused first pass over ot0+ot1, interleaved kp-major across
                # both psum generations: while the x stream arrives, every
                # landed k-pair feeds 8 matmuls instead of 4, so the PE
                # trails the DMA instead of stalling after it.
                pss = [alloc_psums() for _ in range(2)]
                for kp in range(KP):
                    for o in range(2):
                        for bt in range(BT):
                            nc.tensor.matmul(
                                pss[o][bt][:],
                                lhsT=swts[o][:, 2 * kp : 2 * kp + 2, :],
                                rhs=sxT[:, 2 * kp : 2 * kp + 2, ts(bt, NB)],
                                start=(kp == 0),
                                stop=(kp == KP - 1),
                                perf_mode=mybir.MatmulPerfMode.DoubleRow,
                            )
                return pss

            def mm_drain(ot, psums, defer_casts=False):
                # Drain PSUM on DVE. All bn_stats before all copies: the
                # aggr -> partial-stats -> exchange chain is the tail's
                # critical path. For the very last tile the casts are
                # deferred past the exchange trigger (no matmuls need
                # those banks afterwards).
                st6 = stp.tile([128, BT, 6], f32, tag="st6", name="st6", bufs=4)
                for bt in range(BT):
                    nc.vector.bn_stats(st6[:, bt, :], psums[bt][:])
                aggr = nc.vector.bn_aggr(mvT[:, :, ot], st6[:])
                if not defer_casts:
                    for bt in range(BT):
                        nc.vector.tensor_copy(
                            yTt[:, ot, ts(bt, NB)], psums[bt][:]
                        )
                return aggr, psums

            def mm_tile(ot, swt, defer_casts=False):
                psums = alloc_psums()
                mm_mms(swt, psums)
                return mm_drain(ot, psums, defer_casts=defer_casts)

            def stats_pre(q):
                """Partial [mean/4, E[y^2]/4] -> own slot + trigger the
                pre-generated peer sends, right after group q's matmuls."""
                o0, HOT = GO[q], GS[q]
                osl = slice(o0, o0 + HOT)
                arT = arTs[q]
                tmp = stp.tile([128, HOT], f32, tag="tmp_ar", name="tmp_ar")
                w0 = nc.vector.tensor_scalar_mul(
                    arT[:, 0, :], mvT[:, 0, osl], 1.0 / n_group
                )
                nc.vector.tensor_mul(tmp[:], mvT[:, 0, osl], mvT[:, 0, osl])
                nc.vector.tensor_add(tmp[:], tmp[:], mvT[:, 1, osl])
                w1 = nc.vector.tensor_scalar_mul(
                    arT[:, 1, :], tmp[:], 1.0 / n_group
                )
                # The preps were desc-generated long before arT exists, so
                # Tile has no producer to defer the source read against —
                # gate the trigger on the arT writes explicitly (without
                # this the sends fire at ~20us with garbage). A dedicated
                # DVE sem_inc right after the writes gives the GpSimd
                # trigger a precise signal (a plain dep edge resolves via
                # Tile's periodic engine ticks — ~9us of slack, measured).
                si = nc.vector.sem_inc(tsem, 1)
                add_dep_helper(si.ins, w0.ins, sync=False,
                               reason="arT-ready inc after mean write")
                add_dep_helper(si.ins, w1.ins, sync=False,
                               reason="arT-ready inc after sumsq write")
                own = nc.vector.tensor_copy(grAs[q][:, 0, :, :], arT[:])
                tw = nc.gpsimd.wait_ge(tsem, q + 1)
                trig = nc.gpsimd.trigger_dma(count=None)
                add_dep_helper(trig.ins, tw.ins, sync=False,
                               reason="trigger after arT-ready wait")
                return own

            def stats_post(q, anchor=None, pre_deps=()):
                """Wait for the three peers' writes, then global stats ->
                scale/bias for group q. The wait head-of-line-blocks the
                in-order DVE queue, so everything that can run now must be
                ordered ahead of it (pre_deps)."""
                o0, HOT = GO[q], GS[q]
                osl = slice(o0, o0 + HOT)
                w = nc.vector.wait_ge(rsem, 2 * (n_group - 1) * (q + 1))
                if anchor is not None:
                    add_dep_helper(w.ins, anchor.ins, sync=False,
                                   reason="rdma wait after local work")
                for d in pre_deps:
                    add_dep_helper(w.ins, d.ins, sync=False,
                                   reason="rdma wait after local work")
                grT = stp.tile([128, 2, HOT], f32, tag="grT", name="grT")
                first = nc.vector.tensor_reduce(
                    grT[:],
                    grAs[q][:].rearrange("p r two h -> p two h r"),
                    axis=mybir.AxisListType.X,
                    op=mybir.AluOpType.add,
                )
                add_dep_helper(first.ins, w.ins, sync=False,
                               reason="reduce after rdma wait")
                gmean = grT[:, 0, :]
                gvar = stp.tile([128, HOT], f32, tag="gvar", name="gvar")
                nc.vector.tensor_mul(gvar[:], gmean, gmean)
                nc.vector.tensor_sub(gvar[:], grT[:, 1, :], gvar[:])
                # sqrt with eps folded into the ACT bias + plain reciprocal,
                # no Newton refine: the approx error is far inside the 2e-2
                # gate and this chain is the exposed post-exchange tail.
                sq = stp.tile([128, HOT], f32, tag="sq", name="sq")
                nc.scalar.activation(sq[:], gvar[:],
                                     mybir.ActivationFunctionType.Sqrt,
                                     epsT[:], 1.0, 0.0)
                r = stp.tile([128, HOT], f32, tag="r", name="rstd")
                nc.vector.reciprocal(r[:], sq[:])
                t2 = stp.tile([128, HOT], f32, tag="t2", name="t2")
                nc.vector.tensor_mul(scal[:, osl], gbt[:, 0, osl], r[:])
                nc.vector.tensor_mul(t2[:], gmean, scal[:, osl])
                nc.vector.tensor_sub(nbias[:, osl], gbt[:, 1, osl], t2[:])

            def norm_group(q):
                # DVE mul-add in fp16; stores alternate ACT/Sync HWDGE.
                last = None
                for ot in range(GO[q], GO[q] + GS[q]):
                    ob = outp.tile([128, Bc], f16, tag="ob", name="ob")
                    last = nc.vector.tensor_scalar(
                        ob[:],
                        yTt[:, ot, :],
                        scal[:, ot : ot + 1],
                        nbias[:, ot : ot + 1],
                        op0=mybir.AluOpType.mult,
                        op1=mybir.AluOpType.add,
                    )
                    eng = nc.scalar if ot % 2 else nc.sync
                    eng.dma_start(out=yt[ts(ot, 128), :], in_=ob[:])
                return last

            # ---- emission order == scheduling priority ----
            # x lands by DMA straight in the standing sxT tile, one chunk
            # per k-PAIR (the DoubleRow consumption unit), on the ACT HWDGE
            # queue; w rides Sync — two queues so neither head-of-line-
            # blocks the other. First matmul needs only the kp0 quarter of
            # w0/w1 plus the x kp0 chunk, so those lead.
            def x_dma(kp):
                nc.scalar.dma_start(
                    out=sxT[:, 2 * kp : 2 * kp + 2, :],
                    in_=xt[ts(kp, 256), :].rearrange("(f p) b -> p f b", p=128),
                )

            def w_piece(swt, ot, h, ck):
                nc.sync.dma_start(
                    out=swt[:, h * ck : (h + 1) * ck, :],
                    in_=w2[ot, :, h * ck : (h + 1) * ck, :],
                )

            def w_dma(ot):
                swt = swp.tile([128, KT, 128], fp8, tag="swt", name="swt")
                hk = KT // 2
                for h in range(2):
                    w_piece(swt, ot, h, hk)
                return swt

            ck4 = KT // 4
            swt0 = swp.tile([128, KT, 128], fp8, tag="swt", name="swt")
            swt1 = swp.tile([128, KT, 128], fp8, tag="swt", name="swt")
            w_piece(swt0, 0, 0, ck4)
            w_piece(swt1, 1, 0, ck4)
            x_dma(0)
            for h in range(1, 4):
                w_piece(swt0, 0, h, ck4)
                w_piece(swt1, 1, h, ck4)
            for kp in range(1, KP):
                x_dma(kp)
            nc.sync.dma_start(out=gbt[:], in_=gb[:])
            nc.gpsimd.collective_compute(
                "AllGather",
                mybir.AluOpType.bypass,
                replica_groups=[list(range(N_CORES))],
                ins=[ccd_in[:]],
                outs=[ccd_out[:]],
            )

            assert GS[0] >= 3, "fused ot0/ot1 pass assumes both in group 0"
            pss01 = mm_fused01([swt0, swt1])
            swt_next = w_dma(2)
            rdma_prep(0)
            aggrs = [mm_drain(0, pss01[0])[0], mm_drain(1, pss01[1])[0]]

            # post(q) anchors: far enough after group q's exchange fired
            # that the peers' writes have certainly landed even with
            # inter-core skew (post(0) sits 5+ tiles past group 0's
            # trigger). A blocked DVE wait at tile t delays only tile
            # t+2's matmuls (psp bufs=2), so post(1)@OT-2 can never gate
            # the PE at all. norm(q) follows post(q).
            post_at = {OT - 4: 0, OT - 2: 1}

            last_tile_psums = None
            norm1_last = None
            for q in range(NSPLIT):
                for ot in range(max(GO[q], 2), GO[q] + GS[q]):
                    swt = swt_next
                    if ot + 1 < OT:
                        swt_next = w_dma(ot + 1)
                    is_last = ot == OT - 1
                    aggr, psums = mm_tile(ot, swt, defer_casts=is_last)
                    aggrs.append(aggr)
                    if is_last:
                        last_tile_psums = psums
                    if ot == GO[q] + GS[q] - 1:
                        own = stats_pre(q)
                        if q + 1 < NSPLIT:
                            rdma_prep(q + 1)
                    pq = post_at.get(ot)
                    if pq is not None:
                        stats_post(pq, anchor=aggrs[ot])
                        norm1_last = norm_group(pq)

            # Tail: the deferred last-tile casts fill the exchange
            # latency; the final wait is ordered after them and after the
            # previous group's normalize so nothing queues behind it.
            casts = []
            for bt in range(BT):
                casts.append(nc.vector.tensor_copy(
                    yTt[:, OT - 1, ts(bt, NB)], last_tile_psums[bt][:]
                ))
            pre = [casts[-1], own]
            if norm1_last is not None:
                pre.append(norm1_last)
            stats_post(NSPLIT - 1, pre_deps=pre)
            norm_group(NSPLIT - 1)

    nc.finalize()
    return nc


def shard_inputs(x, w, gamma, beta, kb=KB_SHARD, ko=KO_SHARD):
    B, IN = x.shape
    OUT = w.shape[0]
    Bc = B // kb
    OUTc = OUT // ko
    KT, OT = IN // 128, OUTc // 128
    e4 = ml_dtypes.float8_e4m3
    # Host-computed sign bytes (exact): x -> +-0.5 (0x30/0xB0),
    # w -> +-1 (0x38/0xB8). signbit(+0.) is False, so sign(0) maps to +,
    # which differs from sign()'s 0 on a measure-zero set of fp32 randn.
    xs = np.where(np.signbit(x), 0xB0, 0x30).astype(np.uint8)
    ws = np.where(np.signbit(w), 0xB8, 0x38).astype(np.uint8)
    xts = []
    for ib in range(kb):
        xts.append(np.ascontiguousarray(
            xs[ib * Bc : (ib + 1) * Bc].T
        ).view(e4))
    wgs = []
    for io in range(ko):
        wsh = ws[io * OUTc : (io + 1) * OUTc]
        w2 = np.ascontiguousarray(
            wsh.reshape(OT, 128, KT, 128).transpose(0, 3, 2, 1)
        ).view(e4)
        gbp = np.ascontiguousarray(np.stack(
            [gamma[io * OUTc : (io + 1) * OUTc].reshape(OT, 128).T,
             beta[io * OUTc : (io + 1) * OUTc].reshape(OT, 128).T],
            axis=1,
        )).astype(np.float32)
        wgs.append((w2, gbp))
    in_maps = []
    for c in range(kb * ko):
        io, ib = c // kb, c % kb
        in_maps.append({"xt": xts[ib], "w2": wgs[io][0], "gb": wgs[io][1]})
    return in_maps


_NC_CACHE = {}


def kernel(x, w, gamma, beta):
    x = np.asarray(x)
    w = np.asarray(w)
    gamma = np.asarray(gamma)
    beta = np.asarray(beta)
    B, IN = x.shape
    OUT = w.shape[0]

    key = (B, IN, OUT)
    if key not in _NC_CACHE:
        _NC_CACHE[key] = build(B, IN, OUT)
    nc = _NC_CACHE[key]

    in_maps = shard_inputs(x, w, gamma, beta)
    res = run_bass_kernel_spmd(nc, in_maps, list(range(N_CORES)))
    Bc, OUTc = B // KB_SHARD, OUT // KO_SHARD
    out = np.empty((B, OUT), np.float32)
    for c in range(N_CORES):
        io, ib = c // KB_SHARD, c % KB_SHARD
        out[ib * Bc : (ib + 1) * Bc, io * OUTc : (io + 1) * OUTc] = (
            res.results[c]["yt"].T.astype(np.float32)
        )
    return out


if __name__ == "__main__":
    rng = np.random.default_rng(0)
    B, IN, OUT = 8192, 4096, 4096
    x = rng.standard_normal((B, IN)).astype(np.float32)
    w = rng.standard_normal((OUT, IN)).astype(np.float32)
    gamma = np.ones(OUT, np.float32)
    beta = np.zeros(OUT, np.float32)
    out = kernel(x, w, gamma, beta)
    print(out.shape, out.dtype)


# revision 19
# speedup vs baseline: 1.0001x; 1.0001x over previous
"""Trainium2 Bass kernel for BinaryLinearUnit:
    y = sign(x) @ sign(w).T ; BatchNorm1d(train) ; * gamma + beta

Strategy: 2D sharding over 8 NeuronCores — 4 batch shards x 2
output-feature shards. Each core computes y.T for its [2048 batch x
2048 out-features] block with an FP8 (DoubleRow) matmul.

Signs are precomputed on the host (sharding-time byte maps, exact):
  - x ships as +-0.5 fp8e4m3 bytes (0x30/0xB0), K-major: 8MB/core.
    BatchNorm cancels any constant scale of y, so +-0.5 == +-1.
  - w ships as +-1 fp8e4m3 bytes (0x38/0xB8), K-major packed: 8MB/core.
  - y_hat ships back as fp16 (~5e-4 rel err): 8MB/core.
PSUM accumulation is fp32-exact (y/2 is a sum of +-0.5 with |y|<=4096,
and y/2 is exact in fp16). No sign ops on device at all — ACT and DVE
are free for BN work, and the first matmul is gated only by the first
w/x DMA chunks.

BN batch stats need cross-core reduction only within each group of 4
cores that shares the same output-feature shard (logical groups
[0-3], [4-7] — XOR-cosets). Instead of collective_compute AllGathers
(~31us each on the CC stream, measured), partial [mean, E[y^2]] tiles
are exchanged with remote_dma_broadcast: each core sends its partial
to peer c^k which lands in slot k of the peer's SBUF landing tile
(XOR symmetry makes the same static APs correct on every core), with
a monotonic-semaphore bump on arrival. Exchange latency is a few us,
so the post-matmul tail is bn_stats + exchange + normalize (~15us)
instead of a ~45us exposed collective. Descriptors are pre-generated
on GpSimd well before each group ends; only trigger_dma is on the
critical path (the Tile-managed count=None path carries the source
read deps).

The output tiles are processed in NSPLIT stat groups; earlier groups'
exchanges + post math + normalization all overlap remaining matmuls.
The PE span is throttle-bound (GPIO 13/16 duty after ~40us — board
power limit, measured via ntff ham records), so the only other levers
are the startup to first matmul and the tail.

Engine assignment: PE matmuls | ACT x-input DMA queue + sqrt + half
the output stores | DVE bn_stats, psum->f16 copy, stats math, rdma
waits, normalize | GpSimd rdma desc-gen + triggers | Sync w/gb DMA +
half the output stores.
"""

import numpy as np
import ml_dtypes

import concourse.bass as bass
import concourse.mybir as mybir
import concourse.tile as tile
import concourse.bass_interp as bass_interp
from concourse import bacc
from concourse.bass import ts, create_sync_update
from concourse.bass_utils import run_bass_kernel_spmd
from concourse.tile_rust import add_dep_helper

N_CORES = 8
KB_SHARD = 4            # batch shards
KO_SHARD = 2            # output-feature shards
BN_EPS = 1e-5

f32 = mybir.dt.float32
f16 = mybir.dt.float16
fp8 = mybir.dt.float8e4

# The Tile scheduling pass runs a single-core no-exec sim in which
# remote-DMA sem increments from peer cores never arrive, so a wait on
# them deadlocks the scheduler. Seed those sems with a huge value in
# the scheduling pass only; on HW the real wait still blocks until the
# peers' writes land.
_SEED_SEMS: list = []
_orig_simulate = bass_interp.CoreSim.simulate


def _sim_with_seed(self, *a, **k):
    if self.is_scheduling_pass():
        for h in _SEED_SEMS:
            self.update_semaphore(
                create_sync_update(h, 1 << 20, skip_validation=True)
            )
    return _orig_simulate(self, *a, **k)


bass_interp.CoreSim.simulate = _sim_with_seed


def build(B, IN, OUT, kb=KB_SHARD, ko=KO_SHARD):
    """Per-core SPMD module. Core c handles batch shard c%kb and
    out-feature shard c//kb. Shapes: x [B, IN], w [OUT, IN]."""
    Bc = B // kb            # batch rows per core
    OUTc = OUT // ko        # out features per core
    KT = IN // 128          # k tiles (contraction)
    KP = KT // 2            # fp8 DoubleRow consumes k-pairs
    OT = OUTc // 128        # output-feature tiles per core
    NB = 512                # matmul free dim / psum bank width
    BT = Bc // NB           # b tiles per core
    n_group = N_CORES // ko  # cores sharing one out-feature shard

    # Stat groups: each group's exchange (a few us) + post + normalize
    # overlap the remaining matmuls; only the last group's chain is an
    # exposed tail, so it is small.
    GS = [7, 7, 2] if OT == 16 else [OT - OT // 2, OT // 2]
    NSPLIT = len(GS)
    GO = [sum(GS[:q]) for q in range(NSPLIT)]

    nc = bacc.Bacc("TRN2", target_bir_lowering=False, debug=False,
                   num_devices=N_CORES, monotonic_sem_count=NSPLIT)

    # Per-core external I/O (host pre-transposed, K-major, pre-signed):
    #   xt[k, b] = sign(x[(c%kb)*Bc + b, k]) * 0.5          fp8e4m3
    #   w2[ot, p, ks, o] = sign(w[(c//kb)*OUTc + ot*128 + o, ks*128 + p])
    #   yt[o, b] = out[(c%kb)*Bc + b, (c//kb)*OUTc + o]     fp16
    xt = nc.dram_tensor("xt", [IN, Bc], fp8, kind="ExternalInput")
    w2 = nc.dram_tensor("w2", [OT, 128, KT, 128], fp8, kind="ExternalInput")
    gb = nc.dram_tensor("gb", [128, 2, OT], f32, kind="ExternalInput")
    yt = nc.dram_tensor("yt", [OUTc, Bc], f16, kind="ExternalOutput")

    # Dummy 8-core collective: its presence makes the runtime build the
    # global comm and align core launches (without any collective the
    # cores free-run after their own input staging, ~1.6ms apart —
    # measured: peers' rdma arrived in ms-late bursts). Nothing consumes
    # its output; it runs on the CC stream concurrent with startup DMA.
    ccd_in = nc.dram_tensor("ccd_in", [128, 1], f32)
    ccd_out = nc.dram_tensor("ccd_out", [N_CORES * 128, 1], f32)

    # One monotonic sem PER STAT GROUP, bumped by peers' remote writes
    # (+2 per peer: 8 slots -> 16//8 increments per real dest). A single
    # shared sem would alias groups: a fast peer's group-q+1 increments
    # can satisfy the group-q threshold while a slow peer's group-q data
    # is still in flight (measured as a ~4% variance error on one core).
    # Same sem nums on every core (SPMD). lsem is the send-side release
    # sem (unused: source tiles are standing, never rewritten).
    rsems = [nc.monotonic_semaphore(q).sem() for q in range(NSPLIT)]
    _SEED_SEMS.clear()
    _SEED_SEMS.extend(rsems)
    lsem = nc.alloc_semaphore("rdma_local")
    # Precise trigger gating: the arT-writing DVE ops bump tsem via
    # then_inc, and the GpSimd trigger waits on it directly. (A plain
    # cross-engine dep edge resolves via Tile's periodic engine-tick
    # sems — measured ~9us of slack on the critical trigger path.)
    tsem = nc.alloc_semaphore("rdma_trig")

    with tile.TileContext(nc) as tc:
        with (
            tc.tile_pool(name="big", bufs=1) as big,
            tc.tile_pool(name="sw", bufs=3) as swp,
            tc.tile_pool(name="ps", bufs=2, space="PSUM") as psp,
            tc.tile_pool(name="st", bufs=2) as stp,
            tc.tile_pool(name="outp", bufs=6) as outp,
        ):
            # Standing tensors
            sxT = big.tile([128, KT, Bc], fp8)          # sign(x)/2, K-major
            yTt = big.tile([128, OT, Bc], f16)          # y.T/2 (exact in fp16)
            mvT = big.tile([128, 2, OT], f32)           # per-core [mean, var]
            gbt = big.tile([128, 2, OT], f32)           # [gamma; beta]
            scal = big.tile([128, OT], f32)             # gamma * rstd
            nbias = big.tile([128, OT], f32)            # beta - mean * scal
            epsT = big.tile([128, 1], f32)              # BN eps / 4 (ACT bias)
            nc.vector.memset(epsT[:], BN_EPS / 4.0)
            # Exchange tiles (standing: remote reads/writes outlive any
            # pool-recycle window Tile could reason about).
            arTs = [big.tile([128, 2, GS[q]], f32, name=f"arT{q}")
                    for q in range(NSPLIT)]
            # grA slot 0 = own partial, slot k = from logical peer c^k.
            grAs = [big.tile([128, n_group, 2, GS[q]], f32, name=f"grA{q}")
                    for q in range(NSPLIT)]

            def rdma_prep(q):
                # Desc-gen for group q's three peer sends, emitted well
                # before the group ends so only trigger_dma is on the
                # critical path. Slot k of the receiver gets the sender
                # at XOR-distance k (same static APs on every core).
                for k in range(1, n_group):
                    rd = [None] * 8
                    rd[k] = (0, k)
                    nc.gpsimd.remote_dma_broadcast(
                        out_ap=grAs[q][:, k, :, :], in_ap=arTs[q][:],
                        remote_sem=rsems[q], local_sem=lsem, rdests=rd,
                    )

            def alloc_psums():
                return [
                    psp.tile([128, NB], f32, tag=f"ps{bt}", name=f"psum{bt}")
                    for bt in range(BT)
                ]

            def mm_mms(swt, psums):
                # kp-outer: each stationary load is reused across BT b-tiles;
                # also consumes the x k-pairs progressively during startup.
                for kp in range(KP):
                    for bt in range(BT):
                        nc.tensor.matmul(
                            psums[bt][:],
                            lhsT=swt[:, 2 * kp : 2 * kp + 2, :],
                            rhs=sxT[:, 2 * kp : 2 * kp + 2, ts(bt, NB)],
                            start=(kp == 0),
                            stop=(kp == KP - 1),
                            perf_mode=mybir.MatmulPerfMode.DoubleRow,
                        )

            def mm_fused01(swts):
                # Fused first pass over ot0+ot1, interleaved kp-major across
                # both psum generations: while the x stream arrives, every
                # landed k-pair feeds 8 matmuls instead of 4, so the PE
                # trails the DMA instead of stalling after it.
                pss = [alloc_psums() for _ in range(2)]
                for kp in range(KP):
                    for o in range(2):
                        for bt in range(BT):
                            nc.tensor.matmul(
                                pss[o][bt][:],
                                lhsT=swts[o][:, 2 * kp : 2 * kp + 2, :],
                                rhs=sxT[:, 2 * kp : 2 * kp + 2, ts(bt, NB)],
                                start=(kp == 0),
                                stop=(kp == KP - 1),
                                perf_mode=mybir.MatmulPerfMode.DoubleRow,
                            )
                return pss

            def mm_drain(ot, psums, defer_casts=False):
                # Drain PSUM on DVE. All bn_stats before all copies: the
                # aggr -> partial-stats -> exchange chain is the tail's
                # critical path. For the very last tile the casts are
                # deferred past the exchange trigger (no matmuls need
                # those banks afterwards).
                st6 = stp.tile([128, BT, 6], f32, tag="st6", name="st6", bufs=4)
                for bt in range(BT):
                    nc.vector.bn_stats(st6[:, bt, :], psums[bt][:])
                aggr = nc.vector.bn_aggr(mvT[:, :, ot], st6[:])
                if not defer_casts:
                    for bt in range(BT):
                        nc.vector.tensor_copy(
                            yTt[:, ot, ts(bt, NB)], psums[bt][:]
                        )
                return aggr, psums

            def mm_tile(ot, swt, defer_casts=False):
                psums = alloc_psums()
                mm_mms(swt, psums)
                return mm_drain(ot, psums, defer_casts=defer_casts)

            def stats_pre(q):
                """Partial [mean/4, E[y^2]/4] -> own slot + trigger the
                pre-generated peer sends, right after group q's matmuls."""
                o0, HOT = GO[q], GS[q]
                osl = slice(o0, o0 + HOT)
                arT = arTs[q]
                tmp = stp.tile([128, HOT], f32, tag="tmp_ar", name="tmp_ar")
                w0 = nc.vector.tensor_scalar_mul(
                    arT[:, 0, :], mvT[:, 0, osl], 1.0 / n_group
                )
                nc.vector.tensor_mul(tmp[:], mvT[:, 0, osl], mvT[:, 0, osl])
                nc.vector.tensor_add(tmp[:], tmp[:], mvT[:, 1, osl])
                w1 = nc.vector.tensor_scalar_mul(
                    arT[:, 1, :], tmp[:], 1.0 / n_group
                )
                # The preps were desc-generated long before arT exists, so
                # Tile has no producer to defer the source read against —
                # gate the trigger on the arT writes explicitly (without
                # this the sends fire at ~20us with garbage). A dedicated
                # DVE sem_inc right after the writes gives the GpSimd
                # trigger a precise signal (a plain dep edge resolves via
                # Tile's periodic engine ticks — ~9us of slack, measured).
                si = nc.vector.sem_inc(tsem, 1)
                add_dep_helper(si.ins, w0.ins, sync=False,
                               reason="arT-ready inc after mean write")
                add_dep_helper(si.ins, w1.ins, sync=False,
                               reason="arT-ready inc after sumsq write")
                own = nc.vector.tensor_copy(grAs[q][:, 0, :, :], arT[:])
                tw = nc.gpsimd.wait_ge(tsem, q + 1)
                trig = nc.gpsimd.trigger_dma(count=None)
                add_dep_helper(trig.ins, tw.ins, sync=False,
                               reason="trigger after arT-ready wait")
                return own

            def stats_post(q, anchor=None, pre_deps=()):
                """Wait for the three peers' writes, then global stats ->
                scale/bias for group q. The wait head-of-line-blocks the
                in-order DVE queue, so everything that can run now must be
                ordered ahead of it (pre_deps)."""
                o0, HOT = GO[q], GS[q]
                osl = slice(o0, o0 + HOT)
                w = nc.vector.wait_ge(rsems[q], 2 * (n_group - 1))
                if anchor is not None:
                    add_dep_helper(w.ins, anchor.ins, sync=False,
                                   reason="rdma wait after local work")
                for d in pre_deps:
                    add_dep_helper(w.ins, d.ins, sync=False,
                                   reason="rdma wait after local work")
                grT = stp.tile([128, 2, HOT], f32, tag="grT", name="grT")
                first = nc.vector.tensor_reduce(
                    grT[:],
                    grAs[q][:].rearrange("p r two h -> p two h r"),
                    axis=mybir.AxisListType.X,
                    op=mybir.AluOpType.add,
                )
                add_dep_helper(first.ins, w.ins, sync=False,
                               reason="reduce after rdma wait")
                gmean = grT[:, 0, :]
                gvar = stp.tile([128, HOT], f32, tag="gvar", name="gvar")
                nc.vector.tensor_mul(gvar[:], gmean, gmean)
                nc.vector.tensor_sub(gvar[:], grT[:, 1, :], gvar[:])
                # sqrt with eps folded into the ACT bias + plain reciprocal,
                # no Newton refine: the approx error is far inside the 2e-2
                # gate and this chain is the exposed post-exchange tail.
                sq = stp.tile([128, HOT], f32, tag="sq", name="sq")
                nc.scalar.activation(sq[:], gvar[:],
                                     mybir.ActivationFunctionType.Sqrt,
                                     epsT[:], 1.0, 0.0)
                r = stp.tile([128, HOT], f32, tag="r", name="rstd")
                nc.vector.reciprocal(r[:], sq[:])
                t2 = stp.tile([128, HOT], f32, tag="t2", name="t2")
                nc.vector.tensor_mul(scal[:, osl], gbt[:, 0, osl], r[:])
                nc.vector.tensor_mul(t2[:], gmean, scal[:, osl])
                nc.vector.tensor_sub(nbias[:, osl], gbt[:, 1, osl], t2[:])

            def norm_group(q):
                # DVE mul-add in fp16; stores alternate ACT/Sync HWDGE.
                last = None
                for ot in range(GO[q], GO[q] + GS[q]):
                    ob = outp.tile([128, Bc], f16, tag="ob", name="ob")
                    last = nc.vector.tensor_scalar(
                        ob[:],
                        yTt[:, ot, :],
                        scal[:, ot : ot + 1],
                        nbias[:, ot : ot + 1],
                        op0=mybir.AluOpType.mult,
                        op1=mybir.AluOpType.add,
                    )
                    eng = nc.scalar if ot % 2 else nc.sync
                    eng.dma_start(out=yt[ts(ot, 128), :], in_=ob[:])
                return last

            # ---- emission order == scheduling priority ----
            # x lands by DMA straight in the standing sxT tile, one chunk
            # per k-PAIR (the DoubleRow consumption unit), on the ACT HWDGE
            # queue; w rides Sync — two queues so neither head-of-line-
            # blocks the other. First matmul needs only the kp0 quarter of
            # w0/w1 plus the x kp0 chunk, so those lead.
            def x_dma(kp):
                nc.scalar.dma_start(
                    out=sxT[:, 2 * kp : 2 * kp + 2, :],
                    in_=xt[ts(kp, 256), :].rearrange("(f p) b -> p f b", p=128),
                )

            def w_piece(swt, ot, h, ck):
                nc.sync.dma_start(
                    out=swt[:, h * ck : (h + 1) * ck, :],
                    in_=w2[ot, :, h * ck : (h + 1) * ck, :],
                )

            def w_dma(ot):
                swt = swp.tile([128, KT, 128], fp8, tag="swt", name="swt")
                hk = KT // 2
                for h in range(2):
                    w_piece(swt, ot, h, hk)
                return swt

            ck4 = KT // 4
            swt0 = swp.tile([128, KT, 128], fp8, tag="swt", name="swt")
            swt1 = swp.tile([128, KT, 128], fp8, tag="swt", name="swt")
            w_piece(swt0, 0, 0, ck4)
            w_piece(swt1, 1, 0, ck4)
            x_dma(0)
            for h in range(1, 4):
                w_piece(swt0, 0, h, ck4)
                w_piece(swt1, 1, h, ck4)
            for kp in range(1, KP):
                x_dma(kp)
            nc.sync.dma_start(out=gbt[:], in_=gb[:])
            nc.gpsimd.collective_compute(
                "AllGather",
                mybir.AluOpType.bypass,
                replica_groups=[list(range(N_CORES))],
                ins=[ccd_in[:]],
                outs=[ccd_out[:]],
            )

            assert GS[0] >= 3, "fused ot0/ot1 pass assumes both in group 0"
            pss01 = mm_fused01([swt0, swt1])
            swt_next = w_dma(2)
            rdma_prep(0)
            aggrs = [mm_drain(0, pss01[0])[0], mm_drain(1, pss01[1])[0]]

            # post(0) anchors mid-loop: 5+ tiles past group 0's trigger,
            # so its wait never actually blocks. post(1) is deferred past
            # the LAST tile's bn_stats -> partials -> trigger chain (a
            # blocked group-1 wait ahead of that chain delayed the final
            # trigger by ~18us, measured); its data typically lands
            # before the last matmul anyway. norm(q) follows post(q).
            post_at = {OT - 4: 0}

            last_tile_psums = None
            norm1_last = None
            for q in range(NSPLIT):
                for ot in range(max(GO[q], 2), GO[q] + GS[q]):
                    swt = swt_next
                    if ot + 1 < OT:
                        swt_next = w_dma(ot + 1)
                    is_last = ot == OT - 1
                    aggr, psums = mm_tile(ot, swt, defer_casts=is_last)
                    aggrs.append(aggr)
                    if is_last:
                        last_tile_psums = psums
                    if ot == GO[q] + GS[q] - 1:
                        own = stats_pre(q)
                        if q + 1 < NSPLIT:
                            rdma_prep(q + 1)
                    pq = post_at.get(ot)
                    if pq is not None:
                        stats_post(pq, anchor=aggrs[ot])
                        norm1_last = norm_group(pq)

            # Tail DVE order: [deferred last-tile casts | group-1 wait,
            # post, norm (its data typically arrived during the loop) |
            # group-2 wait, post, norm]. The group-2 trigger depends only
            # on the partials' sem_inc, so none of this delays the sends.
            casts = []
            for bt in range(BT):
                casts.append(nc.vector.tensor_copy(
                    yTt[:, OT - 1, ts(bt, NB)], last_tile_psums[bt][:]
                ))
            stats_post(1, pre_deps=[casts[-1], own])
            norm1_last = norm_group(1)
            stats_post(NSPLIT - 1, pre_deps=[norm1_last])
            norm_group(NSPLIT - 1)

    nc.finalize()
    return nc


def shard_inputs(x, w, gamma, beta, kb=KB_SHARD, ko=KO_SHARD):
    B, IN = x.shape
    OUT = w.shape[0]
    Bc = B // kb
    OUTc = OUT // ko
    KT, OT = IN // 128, OUTc // 128
    e4 = ml_dtypes.float8_e4m3
    # Host-computed sign bytes (exact): x -> +-0.5 (0x30/0xB0),
    # w -> +-1 (0x38/0xB8). signbit(+0.) is False, so sign(0) maps to +,
    # which differs from sign()'s 0 on a measure-zero set of fp32 randn.
    xs = np.where(np.signbit(x), 0xB0, 0x30).astype(np.uint8)
    ws = np.where(np.signbit(w), 0xB8, 0x38).astype(np.uint8)
    xts = []
    for ib in range(kb):
        xts.append(np.ascontiguousarray(
            xs[ib * Bc : (ib + 1) * Bc].T
        ).view(e4))
    wgs = []
    for io in range(ko):
        wsh = ws[io * OUTc : (io + 1) * OUTc]
        w2 = np.ascontiguousarray(
            wsh.reshape(OT, 128, KT, 128).transpose(0, 3, 2, 1)
        ).view(e4)
        gbp = np.ascontiguousarray(np.stack(
            [gamma[io * OUTc : (io + 1) * OUTc].reshape(OT, 128).T,
             beta[io * OUTc : (io + 1) * OUTc].reshape(OT, 128).T],
            axis=1,
        )).astype(np.float32)
        wgs.append((w2, gbp))
    in_maps = []
    for c in range(kb * ko):
        io, ib = c // kb, c % kb
        in_maps.append({"xt": xts[ib], "w2": wgs[io][0], "gb": wgs[io][1]})
    return in_maps


_NC_CACHE = {}


def kernel(x, w, gamma, beta):
    x = np.asarray(x)
    w = np.asarray(w)
    gamma = np.asarray(gamma)
    beta = np.asarray(beta)
    B, IN = x.shape
    OUT = w.shape[0]

    key = (B, IN, OUT)
    if key not in _NC_CACHE:
        _NC_CACHE[key] = build(B, IN, OUT)
    nc = _NC_CACHE[key]

    in_maps = shard_inputs(x, w, gamma, beta)
    res = run_bass_kernel_spmd(nc, in_maps, list(range(N_CORES)))
    Bc, OUTc = B // KB_SHARD, OUT // KO_SHARD
    out = np.empty((B, OUT), np.float32)
    for c in range(N_CORES):
        io, ib = c // KB_SHARD, c % KB_SHARD
        out[ib * Bc : (ib + 1) * Bc, io * OUTc : (io + 1) * OUTc] = (
            res.results[c]["yt"].T.astype(np.float32)
        )
    return out


if __name__ == "__main__":
    rng = np.random.default_rng(0)
    B, IN, OUT = 8192, 4096, 4096
    x = rng.standard_normal((B, IN)).astype(np.float32)
    w = rng.standard_normal((OUT, IN)).astype(np.float32)
    gamma = np.ones(OUT, np.float32)
    beta = np.zeros(OUT, np.float32)
    out = kernel(x, w, gamma, beta)
    print(out.shape, out.dtype)


# revision 21
# speedup vs baseline: 1.0747x; 1.0746x over previous
"""Trainium2 Bass kernel for BinaryLinearUnit:
    y = sign(x) @ sign(w).T ; BatchNorm1d(train) ; * gamma + beta

Strategy: 2D sharding over 8 NeuronCores — 4 batch shards x 2
output-feature shards. Each core computes y.T for its [2048 batch x
2048 out-features] block with an FP8 (DoubleRow) matmul.

Signs are precomputed on the host (sharding-time byte maps, exact):
  - x ships as +-0.5 fp8e4m3 bytes (0x30/0xB0), K-major: 8MB/core.
    BatchNorm cancels any constant scale of y, so +-0.5 == +-1.
  - w ships as +-1 fp8e4m3 bytes (0x38/0xB8), K-major packed: 8MB/core.
  - y_hat ships back as fp16 (~5e-4 rel err): 8MB/core.
PSUM accumulation is fp32-exact (y/2 is a sum of +-0.5 with |y|<=4096,
and y/2 is exact in fp16). No sign ops on device at all — ACT and DVE
are free for BN work, and the first matmul is gated only by the first
w/x DMA chunks.

BN batch stats need cross-core reduction only within each group of 4
cores that shares the same output-feature shard (logical groups
[0-3], [4-7] — XOR-cosets). Instead of collective_compute AllGathers
(~31us each on the CC stream, measured), partial [mean, E[y^2]] tiles
are exchanged with remote_dma_broadcast: each core sends its partial
to peer c^k which lands in slot k of the peer's SBUF landing tile
(XOR symmetry makes the same static APs correct on every core), with
a monotonic-semaphore bump on arrival. Exchange latency is a few us,
so the post-matmul tail is bn_stats + exchange + normalize (~15us)
instead of a ~45us exposed collective. Descriptors are pre-generated
on GpSimd well before each group ends; only trigger_dma is on the
critical path (the Tile-managed count=None path carries the source
read deps).

The output tiles are processed in NSPLIT stat groups; earlier groups'
exchanges + post math + normalization all overlap remaining matmuls.
The PE span is throttle-bound (GPIO 13/16 duty after ~40us — board
power limit, measured via ntff ham records), so the only other levers
are the startup to first matmul and the tail.

Engine assignment: PE matmuls | ACT x-input DMA queue + sqrt + half
the output stores | DVE bn_stats, psum->f16 copy, stats math, rdma
waits, normalize | GpSimd rdma desc-gen + triggers | Sync w/gb DMA +
half the output stores.
"""

import numpy as np
import ml_dtypes

import concourse.bass as bass
import concourse.mybir as mybir
import concourse.tile as tile
import concourse.bass_interp as bass_interp
from concourse import bacc
from concourse.bass import ts, create_sync_update
from concourse.bass_utils import run_bass_kernel_spmd
from concourse.tile_rust import add_dep_helper

N_CORES = 8
KB_SHARD = 4            # batch shards
KO_SHARD = 2            # output-feature shards
BN_EPS = 1e-5

f32 = mybir.dt.float32
f16 = mybir.dt.float16
fp8 = mybir.dt.float8e4

# The Tile scheduling pass runs a single-core no-exec sim in which
# remote-DMA sem increments from peer cores never arrive, so a wait on
# them deadlocks the scheduler. Seed those sems with a huge value in
# the scheduling pass only; on HW the real wait still blocks until the
# peers' writes land.
_SEED_SEMS: list = []
_orig_simulate = bass_interp.CoreSim.simulate


def _sim_with_seed(self, *a, **k):
    if self.is_scheduling_pass():
        for h in _SEED_SEMS:
            self.update_semaphore(
                create_sync_update(h, 1 << 20, skip_validation=True)
            )
    return _orig_simulate(self, *a, **k)


bass_interp.CoreSim.simulate = _sim_with_seed


def build(B, IN, OUT, kb=KB_SHARD, ko=KO_SHARD):
    """Per-core SPMD module. Core c handles batch shard c%kb and
    out-feature shard c//kb. Shapes: x [B, IN], w [OUT, IN]."""
    Bc = B // kb            # batch rows per core
    OUTc = OUT // ko        # out features per core
    KT = IN // 128          # k tiles (contraction)
    KP = KT // 2            # fp8 DoubleRow consumes k-pairs
    OT = OUTc // 128        # output-feature tiles per core
    NB = 512                # matmul free dim / psum bank width
    BT = Bc // NB           # b tiles per core
    n_group = N_CORES // ko  # cores sharing one out-feature shard

    # Stat groups: each group's exchange (a few us) + post + normalize
    # overlap the remaining matmuls; only the last group's chain is an
    # exposed tail, so it is small.
    GS = [7, 7, 2] if OT == 16 else [OT - OT // 2, OT // 2]
    NSPLIT = len(GS)
    GO = [sum(GS[:q]) for q in range(NSPLIT)]

    nc = bacc.Bacc("TRN2", target_bir_lowering=False, debug=False,
                   num_devices=N_CORES, monotonic_sem_count=NSPLIT)

    # Per-core external I/O (host pre-transposed, K-major, pre-signed):
    #   xt[k, b] = sign(x[(c%kb)*Bc + b, k]) * 0.5          fp8e4m3
    #   w2[ot, p, ks, o] = sign(w[(c//kb)*OUTc + ot*128 + o, ks*128 + p])
    #   yt[o, b] = out[(c%kb)*Bc + b, (c//kb)*OUTc + o]     fp16
    xt = nc.dram_tensor("xt", [IN, Bc], fp8, kind="ExternalInput")
    w2 = nc.dram_tensor("w2", [OT, 128, KT, 128], fp8, kind="ExternalInput")
    gb = nc.dram_tensor("gb", [128, 2, OT], f32, kind="ExternalInput")
    yt = nc.dram_tensor("yt", [OUTc, Bc], f16, kind="ExternalOutput")

    # Dummy 8-core collective: its presence makes the runtime build the
    # global comm and align core launches (without any collective the
    # cores free-run after their own input staging, ~1.6ms apart —
    # measured: peers' rdma arrived in ms-late bursts). Nothing consumes
    # its output; it runs on the CC stream concurrent with startup DMA.
    ccd_in = nc.dram_tensor("ccd_in", [128, 1], f32)
    ccd_out = nc.dram_tensor("ccd_out", [N_CORES * 128, 1], f32)

    # One monotonic sem PER STAT GROUP, bumped by peers' remote writes
    # (+2 per peer: 8 slots -> 16//8 increments per real dest). A single
    # shared sem would alias groups: a fast peer's group-q+1 increments
    # can satisfy the group-q threshold while a slow peer's group-q data
    # is still in flight (measured as a ~4% variance error on one core).
    # Same sem nums on every core (SPMD). lsem is the send-side release
    # sem (unused: source tiles are standing, never rewritten).
    rsems = [nc.monotonic_semaphore(q).sem() for q in range(NSPLIT)]
    _SEED_SEMS.clear()
    _SEED_SEMS.extend(rsems)
    lsem = nc.alloc_semaphore("rdma_local")
    # Precise trigger gating: the arT-writing DVE ops bump tsem via
    # then_inc, and the GpSimd trigger waits on it directly. (A plain
    # cross-engine dep edge resolves via Tile's periodic engine-tick
    # sems — measured ~9us of slack on the critical trigger path.)
    tsem = nc.alloc_semaphore("rdma_trig")

    with tile.TileContext(nc) as tc:
        with (
            tc.tile_pool(name="big", bufs=1) as big,
            tc.tile_pool(name="sw", bufs=3) as swp,
            tc.tile_pool(name="ps", bufs=2, space="PSUM") as psp,
            tc.tile_pool(name="st", bufs=2) as stp,
            tc.tile_pool(name="outp", bufs=6) as outp,
        ):
            # Standing tensors
            sxT = big.tile([128, KT, Bc], fp8)          # sign(x)/2, K-major
            yTt = big.tile([128, OT, Bc], f16)          # y.T/2 (exact in fp16)
            mvT = big.tile([128, 2, OT], f32)           # per-core [mean, var]
            gbt = big.tile([128, 2, OT], f32)           # [gamma; beta]
            scal = big.tile([128, OT], f32)             # gamma * rstd
            nbias = big.tile([128, OT], f32)            # beta - mean * scal
            epsT = big.tile([128, 1], f32)              # BN eps / 4 (ACT bias)
            nc.vector.memset(epsT[:], BN_EPS / 4.0)
            # Exchange tiles (standing: remote reads/writes outlive any
            # pool-recycle window Tile could reason about).
            arTs = [big.tile([128, 2, GS[q]], f32, name=f"arT{q}")
                    for q in range(NSPLIT)]
            # grA slot 0 = own partial, slot k = from logical peer c^k.
            grAs = [big.tile([128, n_group, 2, GS[q]], f32, name=f"grA{q}")
                    for q in range(NSPLIT)]

            def rdma_prep(q):
                # Desc-gen for group q's three peer sends, emitted well
                # before the group ends so only trigger_dma is on the
                # critical path. Slot k of the receiver gets the sender
                # at XOR-distance k (same static APs on every core).
                # The dest is duplicated across all 8 slots so the ucode
                # spreads the per-partition descriptors over all 16 SDMA
                # engines instead of 2 — the drain is descriptor-bound
                # (~16us/send on 2 engines, measured).
                for k in range(1, n_group):
                    nc.gpsimd.remote_dma_broadcast(
                        out_ap=grAs[q][:, k, :, :], in_ap=arTs[q][:],
                        remote_sem=rsems[q], local_sem=lsem,
                        rdests=[(0, k)] * 8,
                    )

            def alloc_psums():
                return [
                    psp.tile([128, NB], f32, tag=f"ps{bt}", name=f"psum{bt}")
                    for bt in range(BT)
                ]

            def mm_mms(swt, psums):
                # kp-outer: each stationary load is reused across BT b-tiles;
                # also consumes the x k-pairs progressively during startup.
                for kp in range(KP):
                    for bt in range(BT):
                        nc.tensor.matmul(
                            psums[bt][:],
                            lhsT=swt[:, 2 * kp : 2 * kp + 2, :],
                            rhs=sxT[:, 2 * kp : 2 * kp + 2, ts(bt, NB)],
                            start=(kp == 0),
                            stop=(kp == KP - 1),
                            perf_mode=mybir.MatmulPerfMode.DoubleRow,
                        )

            def mm_fused01(swts):
                # Fused first pass over ot0+ot1, interleaved kp-major across
                # both psum generations: while the x stream arrives, every
                # landed k-pair feeds 8 matmuls instead of 4, so the PE
                # trails the DMA instead of stalling after it.
                pss = [alloc_psums() for _ in range(2)]
                for kp in range(KP):
                    for o in range(2):
                        for bt in range(BT):
                            nc.tensor.matmul(
                                pss[o][bt][:],
                                lhsT=swts[o][:, 2 * kp : 2 * kp + 2, :],
                                rhs=sxT[:, 2 * kp : 2 * kp + 2, ts(bt, NB)],
                                start=(kp == 0),
                                stop=(kp == KP - 1),
                                perf_mode=mybir.MatmulPerfMode.DoubleRow,
                            )
                return pss

            def mm_drain(ot, psums, defer_casts=False):
                # Drain PSUM on DVE. All bn_stats before all copies: the
                # aggr -> partial-stats -> exchange chain is the tail's
                # critical path. For the very last tile the casts are
                # deferred past the exchange trigger (no matmuls need
                # those banks afterwards).
                st6 = stp.tile([128, BT, 6], f32, tag="st6", name="st6", bufs=4)
                for bt in range(BT):
                    nc.vector.bn_stats(st6[:, bt, :], psums[bt][:])
                aggr = nc.vector.bn_aggr(mvT[:, :, ot], st6[:])
                if not defer_casts:
                    for bt in range(BT):
                        nc.vector.tensor_copy(
                            yTt[:, ot, ts(bt, NB)], psums[bt][:]
                        )
                return aggr, psums

            def mm_tile(ot, swt, defer_casts=False):
                psums = alloc_psums()
                mm_mms(swt, psums)
                return mm_drain(ot, psums, defer_casts=defer_casts)

            def stats_pre(q):
                """Partial [mean/4, E[y^2]/4] -> own slot + trigger the
                pre-generated peer sends, right after group q's matmuls."""
                o0, HOT = GO[q], GS[q]
                osl = slice(o0, o0 + HOT)
                arT = arTs[q]
                tmp = stp.tile([128, HOT], f32, tag="tmp_ar", name="tmp_ar")
                w0 = nc.vector.tensor_scalar_mul(
                    arT[:, 0, :], mvT[:, 0, osl], 1.0 / n_group
                )
                nc.vector.tensor_mul(tmp[:], mvT[:, 0, osl], mvT[:, 0, osl])
                nc.vector.tensor_add(tmp[:], tmp[:], mvT[:, 1, osl])
                w1 = nc.vector.tensor_scalar_mul(
                    arT[:, 1, :], tmp[:], 1.0 / n_group
                )
                # The preps were desc-generated long before arT exists, so
                # Tile has no producer to defer the source read against —
                # gate the trigger on the arT writes explicitly (without
                # this the sends fire at ~20us with garbage). A dedicated
                # DVE sem_inc right after the writes gives the GpSimd
                # trigger a precise signal (a plain dep edge resolves via
                # Tile's periodic engine ticks — ~9us of slack, measured).
                si = nc.vector.sem_inc(tsem, 1)
                add_dep_helper(si.ins, w0.ins, sync=False,
                               reason="arT-ready inc after mean write")
                add_dep_helper(si.ins, w1.ins, sync=False,
                               reason="arT-ready inc after sumsq write")
                own = nc.vector.tensor_copy(grAs[q][:, 0, :, :], arT[:])
                tw = nc.gpsimd.wait_ge(tsem, q + 1)
                trig = nc.gpsimd.trigger_dma(count=None)
                add_dep_helper(trig.ins, tw.ins, sync=False,
                               reason="trigger after arT-ready wait")
                return own

            def stats_post(q, anchor=None, pre_deps=()):
                """Wait for the three peers' writes, then global stats ->
                scale/bias for group q. The wait head-of-line-blocks the
                in-order DVE queue, so everything that can run now must be
                ordered ahead of it (pre_deps)."""
                o0, HOT = GO[q], GS[q]
                osl = slice(o0, o0 + HOT)
                # Each peer's send bumps rsem by 16 (2 per duplicated slot).
                w = nc.vector.wait_ge(rsems[q], 16 * (n_group - 1))
                if anchor is not None:
                    add_dep_helper(w.ins, anchor.ins, sync=False,
                                   reason="rdma wait after local work")
                for d in pre_deps:
                    add_dep_helper(w.ins, d.ins, sync=False,
                                   reason="rdma wait after local work")
                grT = stp.tile([128, 2, HOT], f32, tag="grT", name="grT")
                first = nc.vector.tensor_reduce(
                    grT[:],
                    grAs[q][:].rearrange("p r two h -> p two h r"),
                    axis=mybir.AxisListType.X,
                    op=mybir.AluOpType.add,
                )
                add_dep_helper(first.ins, w.ins, sync=False,
                               reason="reduce after rdma wait")
                gmean = grT[:, 0, :]
                gvar = stp.tile([128, HOT], f32, tag="gvar", name="gvar")
                nc.vector.tensor_mul(gvar[:], gmean, gmean)
                nc.vector.tensor_sub(gvar[:], grT[:, 1, :], gvar[:])
                # sqrt with eps folded into the ACT bias + plain reciprocal,
                # no Newton refine: the approx error is far inside the 2e-2
                # gate and this chain is the exposed post-exchange tail.
                sq = stp.tile([128, HOT], f32, tag="sq", name="sq")
                nc.scalar.activation(sq[:], gvar[:],
                                     mybir.ActivationFunctionType.Sqrt,
                                     epsT[:], 1.0, 0.0)
                r = stp.tile([128, HOT], f32, tag="r", name="rstd")
                nc.vector.reciprocal(r[:], sq[:])
                t2 = stp.tile([128, HOT], f32, tag="t2", name="t2")
                nc.vector.tensor_mul(scal[:, osl], gbt[:, 0, osl], r[:])
                nc.vector.tensor_mul(t2[:], gmean, scal[:, osl])
                nc.vector.tensor_sub(nbias[:, osl], gbt[:, 1, osl], t2[:])

            def norm_group(q):
                # DVE mul-add in fp16; stores alternate ACT/Sync HWDGE.
                last = None
                for ot in range(GO[q], GO[q] + GS[q]):
                    ob = outp.tile([128, Bc], f16, tag="ob", name="ob")
                    last = nc.vector.tensor_scalar(
                        ob[:],
                        yTt[:, ot, :],
                        scal[:, ot : ot + 1],
                        nbias[:, ot : ot + 1],
                        op0=mybir.AluOpType.mult,
                        op1=mybir.AluOpType.add,
                    )
                    eng = nc.scalar if ot % 2 else nc.sync
                    eng.dma_start(out=yt[ts(ot, 128), :], in_=ob[:])
                return last

            # ---- emission order == scheduling priority ----
            # x lands by DMA straight in the standing sxT tile, one chunk
            # per k-PAIR (the DoubleRow consumption unit), on the ACT HWDGE
            # queue; w rides Sync — two queues so neither head-of-line-
            # blocks the other. First matmul needs only the kp0 quarter of
            # w0/w1 plus the x kp0 chunk, so those lead.
            def x_dma(kp):
                nc.scalar.dma_start(
                    out=sxT[:, 2 * kp : 2 * kp + 2, :],
                    in_=xt[ts(kp, 256), :].rearrange("(f p) b -> p f b", p=128),
                )

            def w_piece(swt, ot, h, ck):
                nc.sync.dma_start(
                    out=swt[:, h * ck : (h + 1) * ck, :],
                    in_=w2[ot, :, h * ck : (h + 1) * ck, :],
                )

            def w_dma(ot):
                swt = swp.tile([128, KT, 128], fp8, tag="swt", name="swt")
                hk = KT // 2
                for h in range(2):
                    w_piece(swt, ot, h, hk)
                return swt

            ck4 = KT // 4
            swt0 = swp.tile([128, KT, 128], fp8, tag="swt", name="swt")
            swt1 = swp.tile([128, KT, 128], fp8, tag="swt", name="swt")
            w_piece(swt0, 0, 0, ck4)
            w_piece(swt1, 1, 0, ck4)
            x_dma(0)
            for h in range(1, 4):
                w_piece(swt0, 0, h, ck4)
                w_piece(swt1, 1, h, ck4)
            for kp in range(1, KP):
                x_dma(kp)
            nc.sync.dma_start(out=gbt[:], in_=gb[:])
            nc.gpsimd.collective_compute(
                "AllGather",
                mybir.AluOpType.bypass,
                replica_groups=[list(range(N_CORES))],
                ins=[ccd_in[:]],
                outs=[ccd_out[:]],
            )

            assert GS[0] >= 3, "fused ot0/ot1 pass assumes both in group 0"
            pss01 = mm_fused01([swt0, swt1])
            swt_next = w_dma(2)
            rdma_prep(0)
            aggrs = [mm_drain(0, pss01[0])[0], mm_drain(1, pss01[1])[0]]

            # post(0) anchors mid-loop: 5+ tiles past group 0's trigger,
            # so its wait never actually blocks. post(1) is deferred past
            # the LAST tile's bn_stats -> partials -> trigger chain (a
            # blocked group-1 wait ahead of that chain delayed the final
            # trigger by ~18us, measured); its data typically lands
            # before the last matmul anyway. norm(q) follows post(q).
            post_at = {OT - 4: 0}

            last_tile_psums = None
            norm1_last = None
            for q in range(NSPLIT):
                for ot in range(max(GO[q], 2), GO[q] + GS[q]):
                    swt = swt_next
                    if ot + 1 < OT:
                        swt_next = w_dma(ot + 1)
                    is_last = ot == OT - 1
                    aggr, psums = mm_tile(ot, swt, defer_casts=is_last)
                    aggrs.append(aggr)
                    if is_last:
                        last_tile_psums = psums
                    if ot == GO[q] + GS[q] - 1:
                        own = stats_pre(q)
                        if q + 1 < NSPLIT:
                            rdma_prep(q + 1)
                    pq = post_at.get(ot)
                    if pq is not None:
                        stats_post(pq, anchor=aggrs[ot])
                        norm1_last = norm_group(pq)

            # Tail DVE order: [deferred last-tile casts | group-1 wait,
            # post, norm (its data typically arrived during the loop) |
            # group-2 wait, post, norm]. The group-2 trigger depends only
            # on the partials' sem_inc, so none of this delays the sends.
            casts = []
            for bt in range(BT):
                casts.append(nc.vector.tensor_copy(
                    yTt[:, OT - 1, ts(bt, NB)], last_tile_psums[bt][:]
                ))
            stats_post(1, pre_deps=[casts[-1], own])
            norm1_last = norm_group(1)
            stats_post(NSPLIT - 1, pre_deps=[norm1_last])
            norm_group(NSPLIT - 1)

    nc.finalize()
    return nc


def shard_inputs(x, w, gamma, beta, kb=KB_SHARD, ko=KO_SHARD):
    B, IN = x.shape
    OUT = w.shape[0]
    Bc = B // kb
    OUTc = OUT // ko
    KT, OT = IN // 128, OUTc // 128
    e4 = ml_dtypes.float8_e4m3
    # Host-computed sign bytes (exact): x -> +-0.5 (0x30/0xB0),
    # w -> +-1 (0x38/0xB8). signbit(+0.) is False, so sign(0) maps to +,
    # which differs from sign()'s 0 on a measure-zero set of fp32 randn.
    xs = np.where(np.signbit(x), 0xB0, 0x30).astype(np.uint8)
    ws = np.where(np.signbit(w), 0xB8, 0x38).astype(np.uint8)
    xts = []
    for ib in range(kb):
        xts.append(np.ascontiguousarray(
            xs[ib * Bc : (ib + 1) * Bc].T
        ).view(e4))
    wgs = []
    for io in range(ko):
        wsh = ws[io * OUTc : (io + 1) * OUTc]
        w2 = np.ascontiguousarray(
            wsh.reshape(OT, 128, KT, 128).transpose(0, 3, 2, 1)
        ).view(e4)
        gbp = np.ascontiguousarray(np.stack(
            [gamma[io * OUTc : (io + 1) * OUTc].reshape(OT, 128).T,
             beta[io * OUTc : (io + 1) * OUTc].reshape(OT, 128).T],
            axis=1,
        )).astype(np.float32)
        wgs.append((w2, gbp))
    in_maps = []
    for c in range(kb * ko):
        io, ib = c // kb, c % kb
        in_maps.append({"xt": xts[ib], "w2": wgs[io][0], "gb": wgs[io][1]})
    return in_maps


_NC_CACHE = {}


def kernel(x, w, gamma, beta):
    x = np.asarray(x)
    w = np.asarray(w)
    gamma = np.asarray(gamma)
    beta = np.asarray(beta)
    B, IN = x.shape
    OUT = w.shape[0]

    key = (B, IN, OUT)
    if key not in _NC_CACHE:
        _NC_CACHE[key] = build(B, IN, OUT)
    nc = _NC_CACHE[key]

    in_maps = shard_inputs(x, w, gamma, beta)
    res = run_bass_kernel_spmd(nc, in_maps, list(range(N_CORES)))
    Bc, OUTc = B // KB_SHARD, OUT // KO_SHARD
    out = np.empty((B, OUT), np.float32)
    for c in range(N_CORES):
        io, ib = c // KB_SHARD, c % KB_SHARD
        out[ib * Bc : (ib + 1) * Bc, io * OUTc : (io + 1) * OUTc] = (
            res.results[c]["yt"].T.astype(np.float32)
        )
    return out


if __name__ == "__main__":
    rng = np.random.default_rng(0)
    B, IN, OUT = 8192, 4096, 4096
    x = rng.standard_normal((B, IN)).astype(np.float32)
    w = rng.standard_normal((OUT, IN)).astype(np.float32)
    gamma = np.ones(OUT, np.float32)
    beta = np.zeros(OUT, np.float32)
    out = kernel(x, w, gamma, beta)
    print(out.shape, out.dtype)


# revision 26
# speedup vs baseline: 1.0933x; 1.0173x over previous
"""Trainium2 Bass kernel for BinaryLinearUnit:
    y = sign(x) @ sign(w).T ; BatchNorm1d(train) ; * gamma + beta

Strategy: 2D sharding over 8 NeuronCores — 4 batch shards x 2
output-feature shards. Each core computes y.T for its [2048 batch x
2048 out-features] block with an FP8 (DoubleRow) matmul.

Signs are precomputed on the host (sharding-time byte maps, exact):
  - x ships as +-0.5 fp8e4m3 bytes (0x30/0xB0), K-major: 8MB/core.
    BatchNorm cancels any constant scale of y, so +-0.5 == +-1.
  - w ships as +-1 fp8e4m3 bytes (0x38/0xB8), K-major packed: 8MB/core.
  - y_hat ships back as fp16 (~5e-4 rel err): 8MB/core.
PSUM accumulation is fp32-exact (y/2 is a sum of +-0.5 with |y|<=4096,
and y/2 is exact in fp16). No sign ops on device at all — ACT and DVE
are free for BN work, and the first matmul is gated only by the first
w/x DMA chunks.

BN batch stats need cross-core reduction only within each group of 4
cores that shares the same output-feature shard (logical groups
[0-3], [4-7] — XOR-cosets). Instead of collective_compute AllGathers
(~31us each on the CC stream, measured), partial [mean, E[y^2]] tiles
are exchanged with remote_dma_broadcast: each core sends its partial
to peer c^k which lands in slot k of the peer's SBUF landing tile
(XOR symmetry makes the same static APs correct on every core), with
a monotonic-semaphore bump on arrival. Exchange latency is a few us,
so the post-matmul tail is bn_stats + exchange + normalize (~15us)
instead of a ~45us exposed collective. Descriptors are pre-generated
on GpSimd well before each group ends; only trigger_dma is on the
critical path (the Tile-managed count=None path carries the source
read deps).

The output tiles are processed in NSPLIT stat groups; earlier groups'
exchanges + post math + normalization all overlap remaining matmuls.
The PE span is throttle-bound (GPIO 13/16 duty after ~40us — board
power limit, measured via ntff ham records), so the only other levers
are the startup to first matmul and the tail.

Engine assignment: PE matmuls | ACT x-input DMA queue + sqrt + half
the output stores | DVE bn_stats, psum->f16 copy, stats math, rdma
waits, normalize | GpSimd rdma desc-gen + triggers | Sync w/gb DMA +
half the output stores.
"""

import numpy as np
import ml_dtypes

import concourse.bass as bass
import concourse.mybir as mybir
import concourse.tile as tile
import concourse.bass_interp as bass_interp
from concourse import bacc
from concourse.bass import ts, create_sync_update
from concourse.bass_utils import run_bass_kernel_spmd
from concourse.tile_rust import add_dep_helper

N_CORES = 8
KB_SHARD = 4            # batch shards
KO_SHARD = 2            # output-feature shards
BN_EPS = 1e-5

f32 = mybir.dt.float32
f16 = mybir.dt.float16
fp8 = mybir.dt.float8e4

# The Tile scheduling pass runs a single-core no-exec sim in which
# remote-DMA sem increments from peer cores never arrive, so a wait on
# them deadlocks the scheduler. Seed those sems with a huge value in
# the scheduling pass only; on HW the real wait still blocks until the
# peers' writes land.
_SEED_SEMS: list = []
_orig_simulate = bass_interp.CoreSim.simulate


def _sim_with_seed(self, *a, **k):
    if self.is_scheduling_pass():
        for h in _SEED_SEMS:
            self.update_semaphore(
                create_sync_update(h, 1 << 20, skip_validation=True)
            )
    return _orig_simulate(self, *a, **k)


bass_interp.CoreSim.simulate = _sim_with_seed


def build(B, IN, OUT, kb=KB_SHARD, ko=KO_SHARD):
    """Per-core SPMD module. Core c handles batch shard c%kb and
    out-feature shard c//kb. Shapes: x [B, IN], w [OUT, IN]."""
    Bc = B // kb            # batch rows per core
    OUTc = OUT // ko        # out features per core
    KT = IN // 128          # k tiles (contraction)
    KP = KT // 2            # fp8 DoubleRow consumes k-pairs
    OT = OUTc // 128        # output-feature tiles per core
    NB = 512                # matmul free dim / psum bank width
    BT = Bc // NB           # b tiles per core
    n_group = N_CORES // ko  # cores sharing one out-feature shard

    # Stat groups: each group's exchange (a few us) + post + normalize
    # overlap the remaining matmuls; only the last group's chain is an
    # exposed tail, so it is small.
    GS = [7, 7, 2] if OT == 16 else [OT - OT // 2, OT // 2]
    NSPLIT = len(GS)
    GO = [sum(GS[:q]) for q in range(NSPLIT)]

    nc = bacc.Bacc("TRN2", target_bir_lowering=False, debug=False,
                   num_devices=N_CORES, monotonic_sem_count=NSPLIT)

    # Per-core external I/O (host pre-transposed, K-major, pre-signed):
    #   xt[k, b] = sign(x[(c%kb)*Bc + b, k]) * 0.5          fp8e4m3
    #   w2[ot, p, ks, o] = sign(w[(c//kb)*OUTc + ot*128 + o, ks*128 + p])
    #   yt[o, b] = out[(c%kb)*Bc + b, (c//kb)*OUTc + o]     fp16
    xt = nc.dram_tensor("xt", [IN, Bc], fp8, kind="ExternalInput")
    w2 = nc.dram_tensor("w2", [OT, 128, KT, 128], fp8, kind="ExternalInput")
    gb = nc.dram_tensor("gb", [128, 2, OT], f32, kind="ExternalInput")
    yt = nc.dram_tensor("yt", [OUTc, Bc], f16, kind="ExternalOutput")

    # Dummy 8-core collective: its presence makes the runtime build the
    # global comm and align core launches (without any collective the
    # cores free-run after their own input staging, ~1.6ms apart —
    # measured: peers' rdma arrived in ms-late bursts). Nothing consumes
    # its output; it runs on the CC stream concurrent with startup DMA.
    ccd_in = nc.dram_tensor("ccd_in", [128, 1], f32)
    ccd_out = nc.dram_tensor("ccd_out", [N_CORES * 128, 1], f32)

    # One monotonic sem PER STAT GROUP, bumped by peers' remote writes
    # (+2 per peer: 8 slots -> 16//8 increments per real dest). A single
    # shared sem would alias groups: a fast peer's group-q+1 increments
    # can satisfy the group-q threshold while a slow peer's group-q data
    # is still in flight (measured as a ~4% variance error on one core).
    # Same sem nums on every core (SPMD). lsem is the send-side release
    # sem (unused: source tiles are standing, never rewritten).
    rsems = [nc.monotonic_semaphore(q).sem() for q in range(NSPLIT)]
    _SEED_SEMS.clear()
    _SEED_SEMS.extend(rsems)
    lsem = nc.alloc_semaphore("rdma_local")
    # Precise trigger gating: the arT-writing DVE ops bump tsem via
    # then_inc, and the GpSimd trigger waits on it directly. (A plain
    # cross-engine dep edge resolves via Tile's periodic engine-tick
    # sems — measured ~9us of slack on the critical trigger path.)
    tsem = nc.alloc_semaphore("rdma_trig")

    with tile.TileContext(nc) as tc:
        with (
            tc.tile_pool(name="big", bufs=1) as big,
            tc.tile_pool(name="sw", bufs=3) as swp,
            tc.tile_pool(name="ps", bufs=2, space="PSUM") as psp,
            tc.tile_pool(name="st", bufs=2) as stp,
            tc.tile_pool(name="outp", bufs=6) as outp,
        ):
            # Standing tensors
            sxT = big.tile([128, KT, Bc], fp8)          # sign(x)/2, K-major
            yTt = big.tile([128, OT, Bc], f16)          # y.T/2 (exact in fp16)
            mvT = big.tile([128, 2, OT], f32)           # per-core [mean, var]
            gbt = big.tile([128, 2, OT], f32)           # [gamma; beta]
            scal = big.tile([128, OT], f32)             # gamma * rstd
            nbias = big.tile([128, OT], f32)            # beta - mean * scal
            epsT = big.tile([128, 1], f32)              # BN eps / 4 (ACT bias)
            nc.vector.memset(epsT[:], BN_EPS / 4.0)
            # Exchange tiles (standing: remote reads/writes outlive any
            # pool-recycle window Tile could reason about).
            arTs = [big.tile([128, 2, GS[q]], f32, name=f"arT{q}")
                    for q in range(NSPLIT)]
            # grA slot 0 = own partial, slot k = from logical peer c^k.
            grAs = [big.tile([128, n_group, 2, GS[q]], f32, name=f"grA{q}")
                    for q in range(NSPLIT)]

            def rdma_prep(q):
                # Desc-gen for group q's three peer sends, emitted well
                # before the group ends so only trigger_dma is on the
                # critical path. Slot k of the receiver gets the sender
                # at XOR-distance k (same static APs on every core).
                # The dest is duplicated across all 8 slots so the ucode
                # spreads the per-partition descriptors over all 16 SDMA
                # engines instead of 2 — the drain is descriptor-bound
                # (~16us/send on 2 engines, measured).
                for k in range(1, n_group):
                    nc.gpsimd.remote_dma_broadcast(
                        out_ap=grAs[q][:, k, :, :], in_ap=arTs[q][:],
                        remote_sem=rsems[q], local_sem=lsem,
                        rdests=[(0, k)] * 8,
                    )

            def alloc_psums():
                return [
                    psp.tile([128, NB], f32, tag=f"ps{bt}", name=f"psum{bt}")
                    for bt in range(BT)
                ]

            def mm_mms(swt, psums):
                # kp-outer: each stationary load is reused across BT b-tiles;
                # also consumes the x k-pairs progressively during startup.
                for kp in range(KP):
                    for bt in range(BT):
                        nc.tensor.matmul(
                            psums[bt][:],
                            lhsT=swt[:, 2 * kp : 2 * kp + 2, :],
                            rhs=sxT[:, 2 * kp : 2 * kp + 2, ts(bt, NB)],
                            start=(kp == 0),
                            stop=(kp == KP - 1),
                            perf_mode=mybir.MatmulPerfMode.DoubleRow,
                        )

            def mm_fused01(swts):
                # Fused first pass over ot0+ot1, interleaved kp-major across
                # both psum generations: while the x stream arrives, every
                # landed k-pair feeds 8 matmuls instead of 4, so the PE
                # trails the DMA instead of stalling after it.
                pss = [alloc_psums() for _ in range(2)]
                for kp in range(KP):
                    for o in range(2):
                        for bt in range(BT):
                            nc.tensor.matmul(
                                pss[o][bt][:],
                                lhsT=swts[o][:, 2 * kp : 2 * kp + 2, :],
                                rhs=sxT[:, 2 * kp : 2 * kp + 2, ts(bt, NB)],
                                start=(kp == 0),
                                stop=(kp == KP - 1),
                                perf_mode=mybir.MatmulPerfMode.DoubleRow,
                            )
                return pss

            def mm_drain(ot, psums, defer_casts=False):
                # Drain PSUM on DVE. All bn_stats before all copies: the
                # aggr -> partial-stats -> exchange chain is the tail's
                # critical path. For the very last tile the casts are
                # deferred past the exchange trigger (no matmuls need
                # those banks afterwards).
                st6 = stp.tile([128, BT, 6], f32, tag="st6", name="st6", bufs=4)
                for bt in range(BT):
                    nc.vector.bn_stats(st6[:, bt, :], psums[bt][:])
                aggr = nc.vector.bn_aggr(mvT[:, :, ot], st6[:])
                if not defer_casts:
                    for bt in range(BT):
                        nc.vector.tensor_copy(
                            yTt[:, ot, ts(bt, NB)], psums[bt][:]
                        )
                return aggr, psums

            def mm_tile(ot, swt, defer_casts=False):
                psums = alloc_psums()
                mm_mms(swt, psums)
                return mm_drain(ot, psums, defer_casts=defer_casts)

            def stats_pre(q):
                """Partial [mean/4, E[y^2]/4] -> own slot + trigger the
                pre-generated peer sends, right after group q's matmuls."""
                o0, HOT = GO[q], GS[q]
                osl = slice(o0, o0 + HOT)
                arT = arTs[q]
                tmp = stp.tile([128, HOT], f32, tag="tmp_ar", name="tmp_ar")
                w0 = nc.vector.tensor_scalar_mul(
                    arT[:, 0, :], mvT[:, 0, osl], 1.0 / n_group
                )
                nc.vector.tensor_mul(tmp[:], mvT[:, 0, osl], mvT[:, 0, osl])
                nc.vector.tensor_add(tmp[:], tmp[:], mvT[:, 1, osl])
                w1 = nc.vector.tensor_scalar_mul(
                    arT[:, 1, :], tmp[:], 1.0 / n_group
                )
                # The preps were desc-generated long before arT exists, so
                # Tile has no producer to defer the source read against —
                # gate the trigger on the arT writes explicitly (without
                # this the sends fire at ~20us with garbage). A dedicated
                # DVE sem_inc right after the writes gives the GpSimd
                # trigger a precise signal (a plain dep edge resolves via
                # Tile's periodic engine ticks — ~9us of slack, measured).
                si = nc.vector.sem_inc(tsem, 1)
                add_dep_helper(si.ins, w0.ins, sync=False,
                               reason="arT-ready inc after mean write")
                add_dep_helper(si.ins, w1.ins, sync=False,
                               reason="arT-ready inc after sumsq write")
                own = nc.vector.tensor_copy(grAs[q][:, 0, :, :], arT[:])
                tw = nc.gpsimd.wait_ge(tsem, q + 1)
                trig = nc.gpsimd.trigger_dma(count=None)
                add_dep_helper(trig.ins, tw.ins, sync=False,
                               reason="trigger after arT-ready wait")
                return own

            def stats_post(q, anchor=None, pre_deps=()):
                """Wait for the three peers' writes, then global stats ->
                scale/bias for group q. The wait head-of-line-blocks the
                in-order DVE queue, so everything that can run now must be
                ordered ahead of it (pre_deps)."""
                o0, HOT = GO[q], GS[q]
                osl = slice(o0, o0 + HOT)
                # Each peer's send bumps rsem by 16 (2 per duplicated slot).
                w = nc.vector.wait_ge(rsems[q], 16 * (n_group - 1))
                if anchor is not None:
                    add_dep_helper(w.ins, anchor.ins, sync=False,
                                   reason="rdma wait after local work")
                for d in pre_deps:
                    add_dep_helper(w.ins, d.ins, sync=False,
                                   reason="rdma wait after local work")
                grT = stp.tile([128, 2, HOT], f32, tag="grT", name="grT")
                first = nc.vector.tensor_reduce(
                    grT[:],
                    grAs[q][:].rearrange("p r two h -> p two h r"),
                    axis=mybir.AxisListType.X,
                    op=mybir.AluOpType.add,
                )
                add_dep_helper(first.ins, w.ins, sync=False,
                               reason="reduce after rdma wait")
                gmean = grT[:, 0, :]
                gvar = stp.tile([128, HOT], f32, tag="gvar", name="gvar")
                nc.vector.tensor_mul(gvar[:], gmean, gmean)
                nc.vector.tensor_sub(gvar[:], grT[:, 1, :], gvar[:])
                # sqrt with eps folded into the ACT bias + plain reciprocal,
                # no Newton refine: the approx error is far inside the 2e-2
                # gate and this chain is the exposed post-exchange tail.
                sq = stp.tile([128, HOT], f32, tag="sq", name="sq")
                nc.scalar.activation(sq[:], gvar[:],
                                     mybir.ActivationFunctionType.Sqrt,
                                     epsT[:], 1.0, 0.0)
                r = stp.tile([128, HOT], f32, tag="r", name="rstd")
                nc.vector.reciprocal(r[:], sq[:])
                t2 = stp.tile([128, HOT], f32, tag="t2", name="t2")
                nc.vector.tensor_mul(scal[:, osl], gbt[:, 0, osl], r[:])
                nc.vector.tensor_mul(t2[:], gmean, scal[:, osl])
                nc.vector.tensor_sub(nbias[:, osl], gbt[:, 1, osl], t2[:])

            def norm_group(q):
                # DVE mul-add in fp16; stores alternate ACT/Sync HWDGE.
                last = None
                for ot in range(GO[q], GO[q] + GS[q]):
                    ob = outp.tile([128, Bc], f16, tag="ob", name="ob")
                    last = nc.vector.tensor_scalar(
                        ob[:],
                        yTt[:, ot, :],
                        scal[:, ot : ot + 1],
                        nbias[:, ot : ot + 1],
                        op0=mybir.AluOpType.mult,
                        op1=mybir.AluOpType.add,
                    )
                    eng = nc.scalar if ot % 2 else nc.sync
                    eng.dma_start(out=yt[ts(ot, 128), :], in_=ob[:])
                return last

            # ---- emission order == scheduling priority ----
            # x lands by DMA straight in the standing sxT tile, one chunk
            # per k-PAIR (the DoubleRow consumption unit), on the ACT HWDGE
            # queue; w rides Sync — two queues so neither head-of-line-
            # blocks the other. First matmul needs only the kp0 quarter of
            # w0/w1 plus the x kp0 chunk, so those lead.
            x_dmas = []

            def x_dma(kp):
                x_dmas.append(nc.scalar.dma_start(
                    out=sxT[:, 2 * kp : 2 * kp + 2, :],
                    in_=xt[ts(kp, 256), :].rearrange("(f p) b -> p f b", p=128),
                ))

            def w_piece(swt, ot, h, ck):
                return nc.sync.dma_start(
                    out=swt[:, h * ck : (h + 1) * ck, :],
                    in_=w2[ot, :, h * ck : (h + 1) * ck, :],
                )

            def w_dma(ot, gate_kp=None):
                # The fused first pass consumes x chunks at nearly the
                # wire rate; w2/w3 prefetch racing the x stream starved
                # it (~5us of early PE gaps + a HAM re-throttle). Gate
                # those two prefetches on early x chunks having landed —
                # they are still ready tiles ahead of their matmuls.
                swt = swp.tile([128, KT, 128], fp8, tag="swt", name="swt")
                hk = KT // 2
                for h in range(2):
                    dma = w_piece(swt, ot, h, hk)
                    if gate_kp is not None:
                        add_dep_helper(dma.ins, x_dmas[gate_kp].ins,
                                       sync=True,
                                       reason="w prefetch after x stream")
                return swt

            ck4 = KT // 4
            swt0 = swp.tile([128, KT, 128], fp8, tag="swt", name="swt")
            swt1 = swp.tile([128, KT, 128], fp8, tag="swt", name="swt")
            w_piece(swt0, 0, 0, ck4)
            w_piece(swt1, 1, 0, ck4)
            x_dma(0)
            for h in range(1, 4):
                w_piece(swt0, 0, h, ck4)
                w_piece(swt1, 1, h, ck4)
            for kp in range(1, KP):
                x_dma(kp)
            nc.sync.dma_start(out=gbt[:], in_=gb[:])
            nc.gpsimd.collective_compute(
                "AllGather",
                mybir.AluOpType.bypass,
                replica_groups=[list(range(N_CORES))],
                ins=[ccd_in[:]],
                outs=[ccd_out[:]],
            )

            assert GS[0] >= 3, "fused ot0/ot1 pass assumes both in group 0"
            pss01 = mm_fused01([swt0, swt1])
            swt_next = w_dma(2, gate_kp=min(6, KP - 1))
            rdma_prep(0)
            aggrs = [mm_drain(0, pss01[0])[0], mm_drain(1, pss01[1])[0]]

            # post(0) anchors mid-loop: 5+ tiles past group 0's trigger,
            # so its wait never actually blocks. post(1) is deferred past
            # the LAST tile's bn_stats -> partials -> trigger chain (a
            # blocked group-1 wait ahead of that chain delayed the final
            # trigger by ~18us, measured); its data typically lands
            # before the last matmul anyway. norm(q) follows post(q).
            post_at = {OT - 4: 0}

            last_tile_psums = None
            norm1_last = None
            for q in range(NSPLIT):
                for ot in range(max(GO[q], 2), GO[q] + GS[q]):
                    swt = swt_next
                    if ot + 1 < OT:
                        gate = min(10, KP - 1) if ot + 1 == 3 else None
                        swt_next = w_dma(ot + 1, gate_kp=gate)
                    is_last = ot == OT - 1
                    aggr, psums = mm_tile(ot, swt, defer_casts=is_last)
                    aggrs.append(aggr)
                    if is_last:
                        last_tile_psums = psums
                    if ot == GO[q] + GS[q] - 1:
                        own = stats_pre(q)
                        if q + 1 < NSPLIT:
                            rdma_prep(q + 1)
                    pq = post_at.get(ot)
                    if pq is not None:
                        stats_post(pq, anchor=aggrs[ot])
                        norm1_last = norm_group(pq)

            # Tail DVE order: [deferred last-tile casts | group-1 wait,
            # post, norm (its data typically arrived during the loop) |
            # group-2 wait, post, norm]. The group-2 trigger depends only
            # on the partials' sem_inc, so none of this delays the sends.
            casts = []
            for bt in range(BT):
                casts.append(nc.vector.tensor_copy(
                    yTt[:, OT - 1, ts(bt, NB)], last_tile_psums[bt][:]
                ))
            stats_post(1, pre_deps=[casts[-1], own])
            norm1_last = norm_group(1)
            stats_post(NSPLIT - 1, pre_deps=[norm1_last])
            norm_group(NSPLIT - 1)

    nc.finalize()
    return nc


def shard_inputs(x, w, gamma, beta, kb=KB_SHARD, ko=KO_SHARD):
    B, IN = x.shape
    OUT = w.shape[0]
    Bc = B // kb
    OUTc = OUT // ko
    KT, OT = IN // 128, OUTc // 128
    e4 = ml_dtypes.float8_e4m3
    # Host-computed sign bytes (exact): x -> +-0.5 (0x30/0xB0),
    # w -> +-1 (0x38/0xB8). signbit(+0.) is False, so sign(0) maps to +,
    # which differs from sign()'s 0 on a measure-zero set of fp32 randn.
    xs = np.where(np.signbit(x), 0xB0, 0x30).astype(np.uint8)
    ws = np.where(np.signbit(w), 0xB8, 0x38).astype(np.uint8)
    xts = []
    for ib in range(kb):
        xts.append(np.ascontiguousarray(
            xs[ib * Bc : (ib + 1) * Bc].T
        ).view(e4))
    wgs = []
    for io in range(ko):
        wsh = ws[io * OUTc : (io + 1) * OUTc]
        w2 = np.ascontiguousarray(
            wsh.reshape(OT, 128, KT, 128).transpose(0, 3, 2, 1)
        ).view(e4)
        gbp = np.ascontiguousarray(np.stack(
            [gamma[io * OUTc : (io + 1) * OUTc].reshape(OT, 128).T,
             beta[io * OUTc : (io + 1) * OUTc].reshape(OT, 128).T],
            axis=1,
        )).astype(np.float32)
        wgs.append((w2, gbp))
    in_maps = []
    for c in range(kb * ko):
        io, ib = c // kb, c % kb
        in_maps.append({"xt": xts[ib], "w2": wgs[io][0], "gb": wgs[io][1]})
    return in_maps


_NC_CACHE = {}


def kernel(x, w, gamma, beta):
    x = np.asarray(x)
    w = np.asarray(w)
    gamma = np.asarray(gamma)
    beta = np.asarray(beta)
    B, IN = x.shape
    OUT = w.shape[0]

    key = (B, IN, OUT)
    if key not in _NC_CACHE:
        _NC_CACHE[key] = build(B, IN, OUT)
    nc = _NC_CACHE[key]

    in_maps = shard_inputs(x, w, gamma, beta)
    res = run_bass_kernel_spmd(nc, in_maps, list(range(N_CORES)))
    Bc, OUTc = B // KB_SHARD, OUT // KO_SHARD
    out = np.empty((B, OUT), np.float32)
    for c in range(N_CORES):
        io, ib = c // KB_SHARD, c % KB_SHARD
        out[ib * Bc : (ib + 1) * Bc, io * OUTc : (io + 1) * OUTc] = (
            res.results[c]["yt"].T.astype(np.float32)
        )
    return out


if __name__ == "__main__":
    rng = np.random.default_rng(0)
    B, IN, OUT = 8192, 4096, 4096
    x = rng.standard_normal((B, IN)).astype(np.float32)
    w = rng.standard_normal((OUT, IN)).astype(np.float32)
    gamma = np.ones(OUT, np.float32)
    beta = np.zeros(OUT, np.float32)
    out = kernel(x, w, gamma, beta)
    print(out.shape, out.dtype)
